# revision 9
# baseline (speedup 1.0000x reference)
"""2-layer GCN on 8 Trainium2 NeuronCores (Bass).

Strategy (graph/data parallel, per the sharding hint):
  - Nodes sharded 8 ways (2500/core). W1/W2 replicated.
  - coef[e] = dinv[src]*dinv[dst] factors into per-node pre/post scalings,
    so aggregation is a pure segment-sum of u = dinv * (x@W1) rows.
  - Per layer: GEMM on the local shard -> AllGather the scaled node table
    (bf16) to HBM -> dma_gather (SWDGE token-routing DMA) stages each
    dst-block's edge rows into SBUF [128, chunks, f] -> TensorE incidence
    matmuls (one-hot built on DVE via iota/is_equal) segment-sum into fp32
    PSUM -> epilogue adds the self-loop term (dinv^2 * h) and bias.
  - Layer 2 runs at f=40 (aggregating h1@W2 instead of h1, since A and W2
    commute), with rows padded to 128 bf16 = 256B for the gather DMA.

Edge preprocessing (sorting by dst block, padding to 128-edge chunks,
degree/dinv computation, index wrapping) happens host-side in numpy; all
FLOPs on x/W run on the NeuronCores.
"""

import math
import os

import numpy as np

N, F_IN, H, C, E = 20000, 512, 256, 40, 640000
CORES = 8
P = 128
TEMPERATURE = 1.0

_BF16 = None


def _bf16():
    global _BF16
    if _BF16 is None:
        import ml_dtypes

        _BF16 = np.dtype(ml_dtypes.bfloat16)
    return _BF16


# ----------------------------------------------------------------------------
# Host-side preprocessing: shard nodes/edges, build gather indices + one-hot
# index tables, degree normalization.
# ----------------------------------------------------------------------------


def preprocess(x, edge_index, W1, b1, W2, b2, n=N, cores=CORES):
    bf16 = _bf16()
    src = np.asarray(edge_index[0], dtype=np.int64)
    dst = np.asarray(edge_index[1], dtype=np.int64)
    ns = n // cores
    nblk = math.ceil(ns / P)

    deg = np.bincount(dst, minlength=n).astype(np.float32) + 1.0
    dinv = (1.0 / np.sqrt(deg)).astype(np.float32)

    core = dst // ns
    blk = (dst % ns) // P
    dloc_all = (dst % ns) % P
    gkey = core * nblk + blk
    order = np.lexsort((src, gkey))
    src_s, gkey_s, dloc_s = src[order], gkey[order], dloc_all[order]

    counts = np.bincount(gkey_s, minlength=cores * nblk).reshape(cores, nblk)
    caps = np.maximum(np.ceil(counts / P).astype(np.int64).max(axis=0), 1)
    ct = int(caps.sum())  # total chunks per core
    ep = ct * P  # padded edges per core

    gidx = np.zeros((cores, ep), dtype=np.int16)
    dloc = np.full((cores, ep), -1, dtype=np.float32)
    starts = np.concatenate([[0], np.cumsum(caps)]) * P  # per-block offsets
    gstart = np.concatenate([[0], np.cumsum(counts.reshape(-1))])
    for c in range(cores):
        for I in range(nblk):
            g = c * nblk + I
            cnt = counts[c, I]
            s0 = gstart[g]
            o0 = starts[I]
            gidx[c, o0 : o0 + cnt] = src_s[s0 : s0 + cnt].astype(np.int16)
            dloc[c, o0 : o0 + cnt] = dloc_s[s0 : s0 + cnt].astype(np.float32)

    # wrap gather indices: position i -> [i%16, i//16], tiled to 128 parts
    gidx_w = np.ascontiguousarray(
        np.tile(gidx.reshape(cores, ep // 16, 16).transpose(0, 2, 1), (1, 8, 1))
    )
    # one-hot index table: position i -> [i%128, i//128]
    dloc_t = np.ascontiguousarray(
        dloc.reshape(cores, ct, P).transpose(0, 2, 1)
    ).astype(bf16)

    # per-core dinv tables [128, nblk]
    dinv_tab = np.ones((cores, P, nblk), dtype=np.float32)
    dinvsq_tab = np.ones((cores, P, nblk), dtype=np.float32)
    for c in range(cores):
        d = dinv[c * ns : (c + 1) * ns]
        pad = np.ones(nblk * P - ns, dtype=np.float32)
        dp = np.concatenate([d, pad]).reshape(nblk, P).T
        dinv_tab[c] = dp
        dinvsq_tab[c] = dp * dp

    x = np.asarray(x, dtype=np.float32)
    W1b = np.asarray(W1, dtype=np.float32).astype(bf16)
    W2b = np.asarray(W2, dtype=np.float32).astype(bf16)
    b1r = np.broadcast_to(np.asarray(b1, np.float32), (P, W1b.shape[1])).copy()
    b2r = np.broadcast_to(np.asarray(b2, np.float32), (P, W2b.shape[1])).copy()

    in_maps = []
    for c in range(cores):
        in_maps.append(
            {
                "x": np.ascontiguousarray(x[c * ns : (c + 1) * ns]).astype(bf16),
                "w1": W1b,
                "w2": W2b,
                "b1": b1r,
                "b2": b2r,
                "dinv": np.ascontiguousarray(dinv_tab[c]),
                "dinvsq": np.ascontiguousarray(dinvsq_tab[c]),
                "gidx": gidx_w[c],
                "dloc": dloc_t[c],
            }
        )
    layout = {
        "n": n,
        "cores": cores,
        "ns": ns,
        "nblk": nblk,
        "caps": [int(v) for v in caps],
        "ct": ct,
        "ep": ep,
    }
    return in_maps, layout


# ----------------------------------------------------------------------------
# Device kernel builder
# ----------------------------------------------------------------------------


def build(layout, f_in=F_IN, h=H, c_out=C, reps=1):
    from concourse import bacc, mybir, tile
    from concourse.masks import make_identity

    n = layout["n"]
    cores = layout["cores"]
    ns = layout["ns"]
    nblk = layout["nblk"]
    caps = layout["caps"]
    ct = layout["ct"]
    ep = layout["ep"]
    kc1 = f_in // P
    kc2 = h // P
    zpad = P  # padded layer-2 row width (256B bf16)
    starts = [0]
    for cap in caps:
        starts.append(starts[-1] + cap)

    bf = mybir.dt.bfloat16
    f32 = mybir.dt.float32

    nc = bacc.Bacc(
        "TRN2", target_bir_lowering=False, debug=False, num_devices=cores
    )
    x_in = nc.dram_tensor("x", [ns, f_in], bf, kind="ExternalInput").ap()
    w1_in = nc.dram_tensor("w1", [f_in, h], bf, kind="ExternalInput").ap()
    w2_in = nc.dram_tensor("w2", [h, c_out], bf, kind="ExternalInput").ap()
    b1_in = nc.dram_tensor("b1", [P, h], f32, kind="ExternalInput").ap()
    b2_in = nc.dram_tensor("b2", [P, c_out], f32, kind="ExternalInput").ap()
    dinv_in = nc.dram_tensor("dinv", [P, nblk], f32, kind="ExternalInput").ap()
    dinvsq_in = nc.dram_tensor(
        "dinvsq", [P, nblk], f32, kind="ExternalInput"
    ).ap()
    gidx_in = nc.dram_tensor(
        "gidx", [P, ep // 16], mybir.dt.int16, kind="ExternalInput"
    ).ap()
    dloc_in = nc.dram_tensor("dloc", [P, ct], bf, kind="ExternalInput").ap()
    out_ext = nc.dram_tensor(
        "out", [ns, c_out], f32, kind="ExternalOutput"
    ).ap()

    last_rows = ns - (nblk - 1) * P  # rows in the final (partial) block
    phase = int(os.environ.get("GCN_PHASE", "4"))

    with tile.TileContext(nc) as tc:
        with (
            tc.tile_pool(name="const", bufs=1) as cpool,
            tc.tile_pool(name="xt", bufs=2) as xtp,
            tc.tile_pool(name="oh", bufs=2) as ohp,
            tc.tile_pool(name="stg", bufs=2) as stgp,
            tc.tile_pool(name="pt", bufs=2, space="PSUM") as ptp,
            tc.tile_pool(name="pg", bufs=2, space="PSUM") as pgp,
            tc.tile_pool(name="pa", bufs=2, space="PSUM") as pap,
            tc.tile_pool(name="dram", bufs=1, space="DRAM") as dram,
        ):
            # ---- constants / persistent state ----
            ident = cpool.tile([P, P], bf)
            make_identity(nc, ident[:])
            iota_i = cpool.tile([P, P], mybir.dt.int16)
            nc.gpsimd.iota(iota_i[:], pattern=[[1, P]], channel_multiplier=0)
            iota_b = cpool.tile([P, P], bf)
            nc.vector.tensor_copy(out=iota_b[:], in_=iota_i[:])

            w1_s = cpool.tile([P, kc1, h], bf)
            for k in range(kc1):
                nc.sync.dma_start(w1_s[:, k, :], w1_in[k * P : (k + 1) * P, :])
            w2_s = cpool.tile([P, kc2, c_out], bf)
            for k in range(kc2):
                nc.sync.dma_start(w2_s[:, k, :], w2_in[k * P : (k + 1) * P, :])
            b1_s = cpool.tile([P, h], f32)
            nc.sync.dma_start(b1_s[:], b1_in[:])
            b2_s = cpool.tile([P, c_out], f32)
            nc.sync.dma_start(b2_s[:], b2_in[:])
            dinv_s = cpool.tile([P, nblk], f32)
            nc.sync.dma_start(dinv_s[:], dinv_in[:])
            dinvsq_s = cpool.tile([P, nblk], f32)
            nc.sync.dma_start(dinvsq_s[:], dinvsq_in[:])
            gidx_s = cpool.tile([P, ep // 16], mybir.dt.int16)
            nc.sync.dma_start(gidx_s[:], gidx_in[:])
            dloc_sb = cpool.tile([P, ct], bf)
            nc.sync.dma_start(dloc_sb[:], dloc_in[:])

            x_s = cpool.tile([P, nblk, f_in], bf)
            if last_rows < P:
                nc.vector.memset(x_s[:, nblk - 1, :], 0.0)
            for I in range(nblk):
                rows = P if I < nblk - 1 else last_rows
                nc.sync.dma_start(
                    x_s[0:rows, I, :], x_in[I * P : I * P + rows, :]
                )

            u_s = cpool.tile([P, nblk, h], bf)
            hs_s = cpool.tile([P, nblk, h], bf)
            h1_s = cpool.tile([P, nblk, h], bf)
            h1t_s = cpool.tile([P, kc2, nblk * P], bf)
            zp_s = cpool.tile([P, nblk, zpad], bf)
            nc.vector.memset(zp_s[:], 0.0)
            zs_s = cpool.tile([P, nblk, c_out], bf)
            out_s = cpool.tile([P, nblk, c_out], f32)

            tp_w = max(f_in, kc2 * P)

            def aggregate(table, elem, f_use, post):
                """Per dst-block: gather edge rows, incidence-matmul the
                segment sum into fp32 PSUM, then run post(I, agg_ap)."""
                for I in range(nblk):
                    cap = caps[I]
                    c0 = starts[I]
                    stg = stgp.tile([P, cap, elem], bf, tag=f"stg{elem}")
                    for g0 in range(0, cap * P, 1024):
                        g1 = min(g0 + 1024, cap * P)
                        nc.gpsimd.dma_gather(
                            stg[:, g0 // P : g1 // P, :],
                            table[:],
                            gidx_s[:, c0 * 8 + g0 // 16 : c0 * 8 + g1 // 16],
                            num_idxs=g1 - g0,
                            num_idxs_reg=g1 - g0,
                            elem_size=elem,
                        )
                    oh = ohp.tile([P, cap, P], bf, tag="oh")
                    nc.vector.tensor_tensor(
                        out=oh[:],
                        in0=dloc_sb[:, c0 : c0 + cap].to_broadcast(
                            [P, cap, P]
                        ),
                        in1=iota_b[:].unsqueeze(1).broadcast_to([P, cap, P]),
                        op=mybir.AluOpType.is_equal,
                    )
                    pat = pap.tile([P, h], f32, space="PSUM", tag="a")
                    pa = pat[:, 0:f_use]
                    for t in range(cap):
                        nc.tensor.matmul(
                            pa,
                            lhsT=oh[:, t, :],
                            rhs=stg[:, t, 0:f_use],
                            start=(t == 0),
                            stop=(t == cap - 1),
                        )
                    post(I, pa)

            for rep in range(reps):
                u_bounce = dram.tile([ns, h], bf, name=f"u_bounce{rep}")
                u_table = dram.tile(
                    [n, h], bf, addr_space="Shared", name=f"u_table{rep}"
                )
                z_bounce = dram.tile([ns, zpad], bf, name=f"z_bounce{rep}")
                z_table = dram.tile(
                    [n, zpad], bf, addr_space="Shared", name=f"z_table{rep}"
                )
                # ---- layer 1 GEMM: h_pre = x @ W1 ----
                for I in range(nblk):
                    xt_ps = ptp.tile([P, tp_w], bf, space="PSUM", tag="t")
                    for k in range(kc1):
                        nc.tensor.transpose(
                            out=xt_ps[:, k * P : (k + 1) * P],
                            in_=x_s[:, I, k * P : (k + 1) * P],
                            identity=ident[:],
                        )
                    xt_s = xtp.tile([P, f_in], bf, tag="xt")
                    nc.vector.tensor_copy(
                        out=xt_s[:], in_=xt_ps[:, 0:f_in]
                    )
                    pg = pgp.tile([P, h], f32, space="PSUM", tag="g")
                    for k in range(kc1):
                        nc.tensor.matmul(
                            pg[:],
                            lhsT=xt_s[:, k * P : (k + 1) * P],
                            rhs=w1_s[:, k, :],
                            start=(k == 0),
                            stop=(k == kc1 - 1),
                        )
                    nc.vector.tensor_scalar_mul(
                        u_s[:, I, :], pg[:], dinv_s[:, I : I + 1]
                    )
                    nc.scalar.mul(
                        hs_s[:, I, :], pg[:], dinvsq_s[:, I : I + 1]
                    )

                # ---- AllGather u ----
                if phase >= 2:
                    for I in range(nblk):
                        rows = P if I < nblk - 1 else last_rows
                        nc.sync.dma_start(
                            u_bounce[I * P : I * P + rows, :],
                            u_s[0:rows, I, :],
                        )
                    nc.gpsimd.collective_compute(
                        "AllGather",
                        mybir.AluOpType.bypass,
                        replica_groups=[list(range(cores))],
                        ins=[u_bounce.opt()],
                        outs=[u_table.opt()],
                    )

                # ---- layer 1 aggregation ----
                def post1(I, pa):
                    nc.vector.scalar_tensor_tensor(
                        out=h1_s[:, I, :],
                        in0=pa,
                        scalar=dinv_s[:, I : I + 1],
                        in1=hs_s[:, I, :],
                        op0=mybir.AluOpType.mult,
                        op1=mybir.AluOpType.add,
                    )
                    nc.vector.tensor_tensor(
                        out=h1_s[:, I, :],
                        in0=h1_s[:, I, :],
                        in1=b1_s[:],
                        op=mybir.AluOpType.add,
                    )
                    nc.vector.tensor_scalar_max(
                        h1_s[:, I, :], h1_s[:, I, :], 0.0
                    )

                if phase >= 3:
                    aggregate(u_table, h, h, post1)
                else:
                    nc.vector.memset(h1_s[:], 0.0)

                if phase >= 4:
                    # ---- layer 2 GEMM: z = h1 @ W2, u2 = dinv*z ----
                    for I in range(nblk):
                        ht_ps = ptp.tile([P, tp_w], bf, space="PSUM", tag="t")
                        for k in range(kc2):
                            nc.tensor.transpose(
                                out=ht_ps[:, k * P : (k + 1) * P],
                                in_=h1_s[:, I, k * P : (k + 1) * P],
                                identity=ident[:],
                            )
                        for k in range(kc2):
                            nc.vector.tensor_copy(
                                out=h1t_s[:, k, I * P : (I + 1) * P],
                                in_=ht_ps[:, k * P : (k + 1) * P],
                            )
                    for I in range(nblk):
                        pzt = pgp.tile([P, h], f32, space="PSUM", tag="g")
                        pz = pzt[:, 0:c_out]
                        for k in range(kc2):
                            nc.tensor.matmul(
                                pz,
                                lhsT=h1t_s[:, k, I * P : (I + 1) * P],
                                rhs=w2_s[:, k, :],
                                start=(k == 0),
                                stop=(k == kc2 - 1),
                            )
                        nc.vector.tensor_scalar_mul(
                            zp_s[:, I, 0:c_out], pz, dinv_s[:, I : I + 1]
                        )
                        nc.scalar.mul(
                            zs_s[:, I, :], pz, dinvsq_s[:, I : I + 1]
                        )

                    # ---- AllGather z ----
                    for I in range(nblk):
                        rows = P if I < nblk - 1 else last_rows
                        nc.sync.dma_start(
                            z_bounce[I * P : I * P + rows, :],
                            zp_s[0:rows, I, :],
                        )
                    nc.gpsimd.collective_compute(
                        "AllGather",
                        mybir.AluOpType.bypass,
                        replica_groups=[list(range(cores))],
                        ins=[z_bounce.opt()],
                        outs=[z_table.opt()],
                    )

                    # ---- layer 2 aggregation ----
                    def post2(I, pa):
                        nc.vector.scalar_tensor_tensor(
                            out=out_s[:, I, :],
                            in0=pa,
                            scalar=dinv_s[:, I : I + 1],
                            in1=zs_s[:, I, :],
                            op0=mybir.AluOpType.mult,
                            op1=mybir.AluOpType.add,
                        )
                        nc.vector.tensor_tensor(
                            out=out_s[:, I, :],
                            in0=out_s[:, I, :],
                            in1=b2_s[:],
                            op=mybir.AluOpType.add,
                        )

                    aggregate(z_table, zpad, c_out, post2)
                else:
                    nc.vector.memset(out_s[:], 0.0)
                    if phase >= 3:
                        for I in range(nblk):
                            nc.vector.tensor_copy(
                                out=out_s[:, I, :], in_=h1_s[:, I, 0:c_out]
                            )
                    else:
                        for I in range(nblk):
                            nc.vector.tensor_copy(
                                out=out_s[:, I, :], in_=u_s[:, I, 0:c_out]
                            )

                # ---- write out ----
                for I in range(nblk):
                    rows = P if I < nblk - 1 else last_rows
                    nc.sync.dma_start(
                        out_ext[I * P : I * P + rows, :], out_s[0:rows, I, :]
                    )

    nc.compile()
    return nc


# ----------------------------------------------------------------------------
# Entry point
# ----------------------------------------------------------------------------

_CACHE = {}


def kernel(x, edge_index, W1, b1, W2, b2):
    in_maps, layout = preprocess(x, edge_index, W1, b1, W2, b2)
    key = (layout["ct"], tuple(layout["caps"]))
    if key not in _CACHE:
        nc = build(layout)
        from runner_inline import SpmdRunner

        _CACHE[key] = (nc, SpmdRunner(nc, layout["cores"]))
    nc, runner = _CACHE[key]
    concat_in = runner.prepare(in_maps)
    out_arrs = runner.run(concat_in)
    res = runner.results(out_arrs)
    ns = layout["ns"]
    out = np.empty((N, C), dtype=np.float32)
    for c in range(layout["cores"]):
        out[c * ns : (c + 1) * ns] = res[c]["out"]
    return out / np.float32(TEMPERATURE)


# Self-contained runner (mirrors concourse.bass2jax.run_bass_via_pjrt but
# keeps the jitted callable for reuse). Written as a sibling module string
# during development; inlined here for the final self-contained kernel.
import sys
import types

_runner_src = None
try:
    import runner as _runner_mod  # dev environment

    sys.modules["runner_inline"] = _runner_mod
except ImportError:  # harness environment: inline definition
    import jax
    from jax.experimental.shard_map import shard_map
    from jax.sharding import Mesh, PartitionSpec

    def _make_runner_module():
        from concourse import bass2jax, mybir
        from concourse.bass2jax import _bass_exec_p, partition_id_tensor

        mod = types.ModuleType("runner_inline")

        class SpmdRunner:
            def __init__(self, nc, n_cores=8):
                bass2jax.install_neuronx_cc_hook()
                self.nc = nc
                self.n_cores = n_cores
                in_names, out_names, out_avals, zero_outs = [], [], [], []
                partition_name = (
                    nc.partition_id_tensor.name
                    if nc.partition_id_tensor
                    else None
                )
                for alloc in nc.m.functions[0].allocations:
                    if not isinstance(alloc, mybir.MemoryLocationSet):
                        continue
                    name = alloc.memorylocations[0].name
                    if alloc.kind == "ExternalInput":
                        if name != partition_name:
                            in_names.append(name)
                    elif alloc.kind == "ExternalOutput":
                        shape = tuple(alloc.tensor_shape)
                        dtype = mybir.dt.np(alloc.dtype)
                        out_names.append(name)
                        out_avals.append(jax.core.ShapedArray(shape, dtype))
                        zero_outs.append(np.zeros(shape, dtype))
                self.in_names = in_names
                self.out_names = out_names
                self.out_avals = out_avals
                n_params = len(in_names)
                n_outs = len(out_avals)
                all_in_names = in_names + out_names
                if partition_name is not None:
                    all_in_names.append(partition_name)

                def _body(*args):
                    operands = list(args)
                    if partition_name is not None:
                        operands.append(partition_id_tensor())
                    outs = _bass_exec_p.bind(
                        *operands,
                        out_avals=tuple(out_avals),
                        in_names=tuple(all_in_names),
                        out_names=tuple(out_names),
                        lowering_input_output_aliases=(),
                        sim_require_finite=True,
                        sim_require_nnan=True,
                        nc=nc,
                    )
                    return tuple(outs)

                devices = jax.devices()[:n_cores]
                mesh = Mesh(np.asarray(devices), ("core",))
                in_specs = (PartitionSpec("core"),) * (n_params + n_outs)
                out_specs = (PartitionSpec("core"),) * n_outs
                self.fn = jax.jit(
                    shard_map(
                        _body,
                        mesh=mesh,
                        in_specs=in_specs,
                        out_specs=out_specs,
                        check_rep=False,
                    ),
                    keep_unused=True,
                )
                self._concat_zeros = [
                    np.zeros((n_cores * z.shape[0], *z.shape[1:]), z.dtype)
                    for z in zero_outs
                ]

            def prepare(self, in_maps):
                per_core = [
                    [np.asarray(m[name]) for name in self.in_names]
                    for m in in_maps
                ]
                return [
                    np.concatenate(
                        [per_core[c][i] for c in range(self.n_cores)], axis=0
                    )
                    for i in range(len(self.in_names))
                ]

            def run(self, concat_in):
                out_arrs = self.fn(*concat_in, *self._concat_zeros)
                jax.block_until_ready(out_arrs)
                return out_arrs

            def results(self, out_arrs):
                return [
                    {
                        name: np.asarray(out_arrs[i]).reshape(
                            self.n_cores, *self.out_avals[i].shape
                        )[c]
                        for i, name in enumerate(self.out_names)
                    }
                    for c in range(self.n_cores)
                ]

        mod.SpmdRunner = SpmdRunner
        return mod

    sys.modules["runner_inline"] = _make_runner_module()


# revision 12
# speedup vs baseline: 1.8931x; 1.8931x over previous
"""2-layer GCN on 8 Trainium2 NeuronCores (Bass).

Strategy (graph/data parallel, per the sharding hint):
  - Nodes sharded 8 ways (2500/core). W1/W2 replicated.
  - coef[e] = dinv[src]*dinv[dst] factors into per-node pre/post scalings,
    so aggregation is a pure segment-sum of u = dinv * (x@W1) rows.
  - Per layer: GEMM on the local shard -> AllGather the scaled node table
    (bf16) to HBM -> dma_gather (SWDGE token-routing DMA) stages each
    dst-block's edge rows into SBUF [128, chunks, f] -> TensorE incidence
    matmuls (one-hot built on DVE via iota/is_equal) segment-sum into fp32
    PSUM -> epilogue adds the self-loop term (dinv^2 * h) and bias.
  - Layer 2 runs at f=40 (aggregating h1@W2 instead of h1, since A and W2
    commute), with rows padded to 128 bf16 = 256B for the gather DMA.

Edge preprocessing (sorting by dst block, padding to 128-edge chunks,
degree/dinv computation, index wrapping) happens host-side in numpy; all
FLOPs on x/W run on the NeuronCores.
"""

import math
import os

import numpy as np

N, F_IN, H, C, E = 20000, 512, 256, 40, 640000
CORES = 8
P = 128
TEMPERATURE = 1.0

_BF16 = None


def _bf16():
    global _BF16
    if _BF16 is None:
        import ml_dtypes

        _BF16 = np.dtype(ml_dtypes.bfloat16)
    return _BF16


# ----------------------------------------------------------------------------
# Host-side preprocessing: shard nodes/edges, build gather indices + one-hot
# index tables, degree normalization.
# ----------------------------------------------------------------------------


def preprocess(x, edge_index, W1, b1, W2, b2, n=N, cores=CORES):
    bf16 = _bf16()
    src = np.asarray(edge_index[0], dtype=np.int64)
    dst = np.asarray(edge_index[1], dtype=np.int64)
    ns = n // cores
    nblk = math.ceil(ns / P)

    deg = np.bincount(dst, minlength=n).astype(np.float32) + 1.0
    dinv = (1.0 / np.sqrt(deg)).astype(np.float32)

    core = dst // ns
    blk = (dst % ns) // P
    dloc_all = (dst % ns) % P
    gkey = core * nblk + blk
    order = np.lexsort((src, gkey))
    src_s, gkey_s, dloc_s = src[order], gkey[order], dloc_all[order]

    counts = np.bincount(gkey_s, minlength=cores * nblk).reshape(cores, nblk)
    caps = np.maximum(np.ceil(counts / P).astype(np.int64).max(axis=0), 1)
    ct = int(caps.sum())  # total chunks per core
    ep = ct * P  # padded edges per core

    gidx = np.zeros((cores, ep), dtype=np.int16)
    dloc = np.full((cores, ep), -1, dtype=np.float32)
    starts = np.concatenate([[0], np.cumsum(caps)]) * P  # per-block offsets
    gstart = np.concatenate([[0], np.cumsum(counts.reshape(-1))])
    for c in range(cores):
        for I in range(nblk):
            g = c * nblk + I
            cnt = counts[c, I]
            s0 = gstart[g]
            o0 = starts[I]
            gidx[c, o0 : o0 + cnt] = src_s[s0 : s0 + cnt].astype(np.int16)
            dloc[c, o0 : o0 + cnt] = dloc_s[s0 : s0 + cnt].astype(np.float32)

    # wrap gather indices: position i -> [i%16, i//16], tiled to 128 parts
    gidx_w = np.ascontiguousarray(
        np.tile(gidx.reshape(cores, ep // 16, 16).transpose(0, 2, 1), (1, 8, 1))
    )
    # one-hot index table: position i -> [i%128, i//128]
    dloc_t = np.ascontiguousarray(
        dloc.reshape(cores, ct, P).transpose(0, 2, 1)
    ).astype(np.float32)

    # per-core dinv tables [128, nblk]
    dinv_tab = np.ones((cores, P, nblk), dtype=np.float32)
    dinvsq_tab = np.ones((cores, P, nblk), dtype=np.float32)
    for c in range(cores):
        d = dinv[c * ns : (c + 1) * ns]
        pad = np.ones(nblk * P - ns, dtype=np.float32)
        dp = np.concatenate([d, pad]).reshape(nblk, P).T
        dinv_tab[c] = dp
        dinvsq_tab[c] = dp * dp

    x = np.asarray(x, dtype=np.float32)
    W1b = np.asarray(W1, dtype=np.float32).astype(bf16)
    W2b = np.asarray(W2, dtype=np.float32).astype(bf16)
    b1r = np.broadcast_to(np.asarray(b1, np.float32), (P, W1b.shape[1])).copy()
    b2r = np.broadcast_to(np.asarray(b2, np.float32), (P, W2b.shape[1])).copy()

    in_maps = []
    for c in range(cores):
        in_maps.append(
            {
                "x": np.ascontiguousarray(x[c * ns : (c + 1) * ns]).astype(bf16),
                "w1": W1b,
                "w2": W2b,
                "b1": b1r,
                "b2": b2r,
                "dinv": np.ascontiguousarray(dinv_tab[c]),
                "dinvsq": np.ascontiguousarray(dinvsq_tab[c]),
                "gidx": gidx_w[c],
                "dloc": dloc_t[c],
            }
        )
    layout = {
        "n": n,
        "cores": cores,
        "ns": ns,
        "nblk": nblk,
        "caps": [int(v) for v in caps],
        "ct": ct,
        "ep": ep,
    }
    return in_maps, layout


# ----------------------------------------------------------------------------
# Device kernel builder
# ----------------------------------------------------------------------------


def build(layout, f_in=F_IN, h=H, c_out=C, reps=1):
    from concourse import bacc, mybir, tile
    from concourse.masks import make_identity

    n = layout["n"]
    cores = layout["cores"]
    ns = layout["ns"]
    nblk = layout["nblk"]
    caps = layout["caps"]
    ct = layout["ct"]
    ep = layout["ep"]
    kc1 = f_in // P
    kc2 = h // P
    zpad = P  # padded layer-2 row width (256B bf16)
    starts = [0]
    for cap in caps:
        starts.append(starts[-1] + cap)

    bf = mybir.dt.bfloat16
    f32 = mybir.dt.float32

    nc = bacc.Bacc(
        "TRN2", target_bir_lowering=False, debug=False, num_devices=cores
    )
    x_in = nc.dram_tensor("x", [ns, f_in], bf, kind="ExternalInput").ap()
    w1_in = nc.dram_tensor("w1", [f_in, h], bf, kind="ExternalInput").ap()
    w2_in = nc.dram_tensor("w2", [h, c_out], bf, kind="ExternalInput").ap()
    b1_in = nc.dram_tensor("b1", [P, h], f32, kind="ExternalInput").ap()
    b2_in = nc.dram_tensor("b2", [P, c_out], f32, kind="ExternalInput").ap()
    dinv_in = nc.dram_tensor("dinv", [P, nblk], f32, kind="ExternalInput").ap()
    dinvsq_in = nc.dram_tensor(
        "dinvsq", [P, nblk], f32, kind="ExternalInput"
    ).ap()
    gidx_in = nc.dram_tensor(
        "gidx", [P, ep // 16], mybir.dt.int16, kind="ExternalInput"
    ).ap()
    dloc_in = nc.dram_tensor("dloc", [P, ct], f32, kind="ExternalInput").ap()
    out_ext = nc.dram_tensor(
        "out", [ns, c_out], f32, kind="ExternalOutput"
    ).ap()

    last_rows = ns - (nblk - 1) * P  # rows in the final (partial) block
    phase = int(os.environ.get("GCN_PHASE", "4"))

    with tile.TileContext(nc) as tc:
        with (
            tc.tile_pool(name="const", bufs=1) as cpool,
            tc.tile_pool(name="xt", bufs=2) as xtp,
            tc.tile_pool(name="oh", bufs=2) as ohp,
            tc.tile_pool(name="stg", bufs=2) as stgp,
            tc.tile_pool(name="pt", bufs=2, space="PSUM") as ptp,
            tc.tile_pool(name="pg", bufs=2, space="PSUM") as pgp,
            tc.tile_pool(name="pa", bufs=2, space="PSUM") as pap,
            tc.tile_pool(name="dram", bufs=1, space="DRAM") as dram,
        ):
            # ---- constants / persistent state ----
            ident = cpool.tile([P, P], bf)
            make_identity(nc, ident[:])
            iota_i = cpool.tile([P, P], mybir.dt.int16)
            nc.gpsimd.iota(iota_i[:], pattern=[[1, P]], channel_multiplier=0)
            iota_b = cpool.tile([P, P], bf)
            nc.vector.tensor_copy(out=iota_b[:], in_=iota_i[:])

            w1_s = cpool.tile([P, kc1, h], bf)
            for k in range(kc1):
                nc.sync.dma_start(w1_s[:, k, :], w1_in[k * P : (k + 1) * P, :])
            w2_s = cpool.tile([P, kc2, c_out], bf)
            for k in range(kc2):
                nc.sync.dma_start(w2_s[:, k, :], w2_in[k * P : (k + 1) * P, :])
            b1_s = cpool.tile([P, h], f32)
            nc.sync.dma_start(b1_s[:], b1_in[:])
            b2_s = cpool.tile([P, c_out], f32)
            nc.sync.dma_start(b2_s[:], b2_in[:])
            dinv_s = cpool.tile([P, nblk], f32)
            nc.sync.dma_start(dinv_s[:], dinv_in[:])
            dinvsq_s = cpool.tile([P, nblk], f32)
            nc.sync.dma_start(dinvsq_s[:], dinvsq_in[:])
            gidx_s = cpool.tile([P, ep // 16], mybir.dt.int16)
            nc.sync.dma_start(gidx_s[:], gidx_in[:])
            dloc_sb = cpool.tile([P, ct], f32)
            nc.sync.dma_start(dloc_sb[:], dloc_in[:])

            x_s = cpool.tile([P, nblk, f_in], bf)
            if last_rows < P:
                nc.vector.memset(x_s[:, nblk - 1, :], 0.0)
            for I in range(nblk):
                rows = P if I < nblk - 1 else last_rows
                nc.sync.dma_start(
                    x_s[0:rows, I, :], x_in[I * P : I * P + rows, :]
                )

            u_s = cpool.tile([P, nblk, h], bf)
            hs_s = cpool.tile([P, nblk, h], bf)
            h1_s = cpool.tile([P, nblk, h], bf)
            h1t_s = cpool.tile([P, kc2, nblk * P], bf)
            zp_s = cpool.tile([P, nblk, zpad], bf)
            nc.vector.memset(zp_s[:], 0.0)
            zs_s = cpool.tile([P, nblk, c_out], bf)
            out_s = cpool.tile([P, nblk, c_out], f32)

            tp_w = max(f_in, kc2 * P)

            agg_mode = os.environ.get("GCN_AGG", "full")

            def aggregate(table, elem, f_use, post):
                """Per dst-block: gather edge rows, incidence-matmul the
                segment sum into fp32 PSUM, then run post(I, agg_ap)."""
                for I in range(nblk):
                    cap = caps[I]
                    c0 = starts[I]
                    stg = stgp.tile([P, cap, elem], bf, tag=f"stg{elem}")
                    spl = int(os.environ.get("GCN_SPL", "768"))
                    for g0 in range(0, cap * P, spl):
                        g1 = min(g0 + spl, cap * P)
                        nc.gpsimd.dma_gather(
                            stg[:, g0 // P : g1 // P, :],
                            table[:],
                            gidx_s[:, c0 * 8 + g0 // 16 : c0 * 8 + g1 // 16],
                            num_idxs=g1 - g0,
                            num_idxs_reg=g1 - g0,
                            elem_size=elem,
                        )
                    pat = pap.tile([P, h], f32, space="PSUM", tag="a")
                    pa = pat[:, 0:f_use]
                    if agg_mode == "gather":
                        nc.tensor.matmul(
                            pa,
                            lhsT=ident[:],
                            rhs=stg[:, 0, 0:f_use],
                            start=True,
                            stop=True,
                        )
                        post(I, pa)
                        continue
                    oh = ohp.tile([P, cap, P], bf, tag="oh")
                    for t in range(cap):
                        nc.vector.tensor_scalar(
                            oh[:, t, :],
                            iota_b[:],
                            dloc_sb[:, c0 + t : c0 + t + 1],
                            None,
                            mybir.AluOpType.is_equal,
                        )
                    if agg_mode == "oh":
                        nc.tensor.matmul(
                            pa,
                            lhsT=oh[:, 0, :],
                            rhs=stg[:, 0, 0:f_use],
                            start=True,
                            stop=True,
                        )
                        post(I, pa)
                        continue
                    for t in range(cap):
                        nc.tensor.matmul(
                            pa,
                            lhsT=oh[:, t, :],
                            rhs=stg[:, t, 0:f_use],
                            start=(t == 0),
                            stop=(t == cap - 1),
                        )
                    post(I, pa)

            for rep in range(reps):
                u_bounce = dram.tile([ns, h], bf, name=f"u_bounce{rep}")
                u_table = dram.tile(
                    [n, h], bf, addr_space="Shared", name=f"u_table{rep}"
                )
                z_bounce = dram.tile([ns, zpad], bf, name=f"z_bounce{rep}")
                z_table = dram.tile(
                    [n, zpad], bf, addr_space="Shared", name=f"z_table{rep}"
                )
                # ---- layer 1 GEMM: h_pre = x @ W1 ----
                for I in range(nblk):
                    xt_ps = ptp.tile([P, tp_w], bf, space="PSUM", tag="t")
                    for k in range(kc1):
                        nc.tensor.transpose(
                            out=xt_ps[:, k * P : (k + 1) * P],
                            in_=x_s[:, I, k * P : (k + 1) * P],
                            identity=ident[:],
                        )
                    xt_s = xtp.tile([P, f_in], bf, tag="xt")
                    nc.vector.tensor_copy(
                        out=xt_s[:], in_=xt_ps[:, 0:f_in]
                    )
                    pg = pgp.tile([P, h], f32, space="PSUM", tag="g")
                    for k in range(kc1):
                        nc.tensor.matmul(
                            pg[:],
                            lhsT=xt_s[:, k * P : (k + 1) * P],
                            rhs=w1_s[:, k, :],
                            start=(k == 0),
                            stop=(k == kc1 - 1),
                        )
                    nc.vector.tensor_scalar_mul(
                        u_s[:, I, :], pg[:], dinv_s[:, I : I + 1]
                    )
                    nc.scalar.mul(
                        hs_s[:, I, :], pg[:], dinvsq_s[:, I : I + 1]
                    )

                # ---- AllGather u ----
                if phase >= 2:
                    for I in range(nblk):
                        rows = P if I < nblk - 1 else last_rows
                        nc.sync.dma_start(
                            u_bounce[I * P : I * P + rows, :],
                            u_s[0:rows, I, :],
                        )
                    nc.gpsimd.collective_compute(
                        "AllGather",
                        mybir.AluOpType.bypass,
                        replica_groups=[list(range(cores))],
                        ins=[u_bounce.opt()],
                        outs=[u_table.opt()],
                    )

                # ---- layer 1 aggregation ----
                def post1(I, pa):
                    nc.vector.scalar_tensor_tensor(
                        out=h1_s[:, I, :],
                        in0=pa,
                        scalar=dinv_s[:, I : I + 1],
                        in1=hs_s[:, I, :],
                        op0=mybir.AluOpType.mult,
                        op1=mybir.AluOpType.add,
                    )
                    nc.vector.tensor_tensor(
                        out=h1_s[:, I, :],
                        in0=h1_s[:, I, :],
                        in1=b1_s[:],
                        op=mybir.AluOpType.add,
                    )
                    nc.vector.tensor_scalar_max(
                        h1_s[:, I, :], h1_s[:, I, :], 0.0
                    )

                if phase >= 3:
                    aggregate(u_table, h, h, post1)
                else:
                    nc.vector.memset(h1_s[:], 0.0)

                if phase >= 4:
                    # ---- layer 2 GEMM: z = h1 @ W2, u2 = dinv*z ----
                    for I in range(nblk):
                        ht_ps = ptp.tile([P, tp_w], bf, space="PSUM", tag="t")
                        for k in range(kc2):
                            nc.tensor.transpose(
                                out=ht_ps[:, k * P : (k + 1) * P],
                                in_=h1_s[:, I, k * P : (k + 1) * P],
                                identity=ident[:],
                            )
                        for k in range(kc2):
                            nc.vector.tensor_copy(
                                out=h1t_s[:, k, I * P : (I + 1) * P],
                                in_=ht_ps[:, k * P : (k + 1) * P],
                            )
                    for I in range(nblk):
                        pzt = pgp.tile([P, h], f32, space="PSUM", tag="g")
                        pz = pzt[:, 0:c_out]
                        for k in range(kc2):
                            nc.tensor.matmul(
                                pz,
                                lhsT=h1t_s[:, k, I * P : (I + 1) * P],
                                rhs=w2_s[:, k, :],
                                start=(k == 0),
                                stop=(k == kc2 - 1),
                            )
                        nc.vector.tensor_scalar_mul(
                            zp_s[:, I, 0:c_out], pz, dinv_s[:, I : I + 1]
                        )
                        nc.scalar.mul(
                            zs_s[:, I, :], pz, dinvsq_s[:, I : I + 1]
                        )

                    # ---- AllGather z ----
                    for I in range(nblk):
                        rows = P if I < nblk - 1 else last_rows
                        nc.sync.dma_start(
                            z_bounce[I * P : I * P + rows, :],
                            zp_s[0:rows, I, :],
                        )
                    nc.gpsimd.collective_compute(
                        "AllGather",
                        mybir.AluOpType.bypass,
                        replica_groups=[list(range(cores))],
                        ins=[z_bounce.opt()],
                        outs=[z_table.opt()],
                    )

                    # ---- layer 2 aggregation ----
                    def post2(I, pa):
                        nc.vector.scalar_tensor_tensor(
                            out=out_s[:, I, :],
                            in0=pa,
                            scalar=dinv_s[:, I : I + 1],
                            in1=zs_s[:, I, :],
                            op0=mybir.AluOpType.mult,
                            op1=mybir.AluOpType.add,
                        )
                        nc.vector.tensor_tensor(
                            out=out_s[:, I, :],
                            in0=out_s[:, I, :],
                            in1=b2_s[:],
                            op=mybir.AluOpType.add,
                        )

                    aggregate(z_table, zpad, c_out, post2)
                else:
                    nc.vector.memset(out_s[:], 0.0)
                    if phase >= 3:
                        for I in range(nblk):
                            nc.vector.tensor_copy(
                                out=out_s[:, I, :], in_=h1_s[:, I, 0:c_out]
                            )
                    else:
                        for I in range(nblk):
                            nc.vector.tensor_copy(
                                out=out_s[:, I, :], in_=u_s[:, I, 0:c_out]
                            )

                # ---- write out ----
                for I in range(nblk):
                    rows = P if I < nblk - 1 else last_rows
                    nc.sync.dma_start(
                        out_ext[I * P : I * P + rows, :], out_s[0:rows, I, :]
                    )

    nc.compile()
    return nc


# ----------------------------------------------------------------------------
# Entry point
# ----------------------------------------------------------------------------

_CACHE = {}


def kernel(x, edge_index, W1, b1, W2, b2):
    in_maps, layout = preprocess(x, edge_index, W1, b1, W2, b2)
    key = (layout["ct"], tuple(layout["caps"]))
    if key not in _CACHE:
        nc = build(layout)
        from runner_inline import SpmdRunner

        _CACHE[key] = (nc, SpmdRunner(nc, layout["cores"]))
    nc, runner = _CACHE[key]
    concat_in = runner.prepare(in_maps)
    out_arrs = runner.run(concat_in)
    res = runner.results(out_arrs)
    ns = layout["ns"]
    out = np.empty((N, C), dtype=np.float32)
    for c in range(layout["cores"]):
        out[c * ns : (c + 1) * ns] = res[c]["out"]
    return out / np.float32(TEMPERATURE)


# Self-contained runner (mirrors concourse.bass2jax.run_bass_via_pjrt but
# keeps the jitted callable for reuse). Written as a sibling module string
# during development; inlined here for the final self-contained kernel.
import sys
import types

_runner_src = None
try:
    import runner as _runner_mod  # dev environment

    sys.modules["runner_inline"] = _runner_mod
except ImportError:  # harness environment: inline definition
    import jax
    from jax.experimental.shard_map import shard_map
    from jax.sharding import Mesh, PartitionSpec

    def _make_runner_module():
        from concourse import bass2jax, mybir
        from concourse.bass2jax import _bass_exec_p, partition_id_tensor

        mod = types.ModuleType("runner_inline")

        class SpmdRunner:
            def __init__(self, nc, n_cores=8):
                bass2jax.install_neuronx_cc_hook()
                self.nc = nc
                self.n_cores = n_cores
                in_names, out_names, out_avals, zero_outs = [], [], [], []
                partition_name = (
                    nc.partition_id_tensor.name
                    if nc.partition_id_tensor
                    else None
                )
                for alloc in nc.m.functions[0].allocations:
                    if not isinstance(alloc, mybir.MemoryLocationSet):
                        continue
                    name = alloc.memorylocations[0].name
                    if alloc.kind == "ExternalInput":
                        if name != partition_name:
                            in_names.append(name)
                    elif alloc.kind == "ExternalOutput":
                        shape = tuple(alloc.tensor_shape)
                        dtype = mybir.dt.np(alloc.dtype)
                        out_names.append(name)
                        out_avals.append(jax.core.ShapedArray(shape, dtype))
                        zero_outs.append(np.zeros(shape, dtype))
                self.in_names = in_names
                self.out_names = out_names
                self.out_avals = out_avals
                n_params = len(in_names)
                n_outs = len(out_avals)
                all_in_names = in_names + out_names
                if partition_name is not None:
                    all_in_names.append(partition_name)

                def _body(*args):
                    operands = list(args)
                    if partition_name is not None:
                        operands.append(partition_id_tensor())
                    outs = _bass_exec_p.bind(
                        *operands,
                        out_avals=tuple(out_avals),
                        in_names=tuple(all_in_names),
                        out_names=tuple(out_names),
                        lowering_input_output_aliases=(),
                        sim_require_finite=True,
                        sim_require_nnan=True,
                        nc=nc,
                    )
                    return tuple(outs)

                devices = jax.devices()[:n_cores]
                mesh = Mesh(np.asarray(devices), ("core",))
                in_specs = (PartitionSpec("core"),) * (n_params + n_outs)
                out_specs = (PartitionSpec("core"),) * n_outs
                self.fn = jax.jit(
                    shard_map(
                        _body,
                        mesh=mesh,
                        in_specs=in_specs,
                        out_specs=out_specs,
                        check_rep=False,
                    ),
                    keep_unused=True,
                )
                self._concat_zeros = [
                    np.zeros((n_cores * z.shape[0], *z.shape[1:]), z.dtype)
                    for z in zero_outs
                ]

            def prepare(self, in_maps):
                per_core = [
                    [np.asarray(m[name]) for name in self.in_names]
                    for m in in_maps
                ]
                return [
                    np.concatenate(
                        [per_core[c][i] for c in range(self.n_cores)], axis=0
                    )
                    for i in range(len(self.in_names))
                ]

            def run(self, concat_in):
                out_arrs = self.fn(*concat_in, *self._concat_zeros)
                jax.block_until_ready(out_arrs)
                return out_arrs

            def results(self, out_arrs):
                return [
                    {
                        name: np.asarray(out_arrs[i]).reshape(
                            self.n_cores, *self.out_avals[i].shape
                        )[c]
                        for i, name in enumerate(self.out_names)
                    }
                    for c in range(self.n_cores)
                ]

        mod.SpmdRunner = SpmdRunner
        return mod

    sys.modules["runner_inline"] = _make_runner_module()


# revision 13
# speedup vs baseline: 8.6572x; 4.5730x over previous
"""2-layer GCN on 8 Trainium2 NeuronCores (Bass).

Strategy (graph/data parallel, per the sharding hint):
  - Nodes sharded 8 ways (2500/core). W1/W2 replicated.
  - coef[e] = dinv[src]*dinv[dst] factors into per-node pre/post scalings,
    so aggregation is a pure segment-sum of u = dinv * (x@W1) rows.
  - Per layer: GEMM on the local shard -> AllGather the scaled node table
    (bf16) to HBM -> dma_gather (SWDGE token-routing DMA) stages each
    dst-block's edge rows into SBUF [128, chunks, f] -> TensorE incidence
    matmuls (one-hot built on DVE via iota/is_equal) segment-sum into fp32
    PSUM -> epilogue adds the self-loop term (dinv^2 * h) and bias.
  - Layer 2 runs at f=40 (aggregating h1@W2 instead of h1, since A and W2
    commute), with rows padded to 128 bf16 = 256B for the gather DMA.

Edge preprocessing (sorting by dst block, padding to 128-edge chunks,
degree/dinv computation, index wrapping) happens host-side in numpy; all
FLOPs on x/W run on the NeuronCores.
"""

import math
import os

import numpy as np

N, F_IN, H, C, E = 20000, 512, 256, 40, 640000
CORES = 8
P = 128
TEMPERATURE = 1.0

_BF16 = None


def _bf16():
    global _BF16
    if _BF16 is None:
        import ml_dtypes

        _BF16 = np.dtype(ml_dtypes.bfloat16)
    return _BF16


# ----------------------------------------------------------------------------
# Host-side preprocessing: shard nodes/edges, build gather indices + one-hot
# index tables, degree normalization.
# ----------------------------------------------------------------------------


def preprocess(x, edge_index, W1, b1, W2, b2, n=N, cores=CORES):
    bf16 = _bf16()
    src = np.asarray(edge_index[0], dtype=np.int64)
    dst = np.asarray(edge_index[1], dtype=np.int64)
    ns = n // cores
    nblk = math.ceil(ns / P)

    deg = np.bincount(dst, minlength=n).astype(np.float32) + 1.0
    dinv = (1.0 / np.sqrt(deg)).astype(np.float32)

    core = dst // ns
    blk = (dst % ns) // P
    dloc_all = (dst % ns) % P
    gkey = core * nblk + blk
    order = np.lexsort((src, gkey))
    src_s, gkey_s, dloc_s = src[order], gkey[order], dloc_all[order]

    counts = np.bincount(gkey_s, minlength=cores * nblk).reshape(cores, nblk)
    caps = np.maximum(np.ceil(counts / P).astype(np.int64).max(axis=0), 1)
    ct = int(caps.sum())  # total chunks per core
    ep = ct * P  # padded edges per core

    gidx = np.zeros((cores, ep), dtype=np.int16)
    dloc = np.full((cores, ep), -1, dtype=np.float32)
    starts = np.concatenate([[0], np.cumsum(caps)]) * P  # per-block offsets
    gstart = np.concatenate([[0], np.cumsum(counts.reshape(-1))])
    for c in range(cores):
        for I in range(nblk):
            g = c * nblk + I
            cnt = counts[c, I]
            s0 = gstart[g]
            o0 = starts[I]
            gidx[c, o0 : o0 + cnt] = src_s[s0 : s0 + cnt].astype(np.int16)
            dloc[c, o0 : o0 + cnt] = dloc_s[s0 : s0 + cnt].astype(np.float32)

    # wrap gather indices: position i -> [i%16, i//16], tiled to 128 parts
    gidx_w = np.ascontiguousarray(
        np.tile(gidx.reshape(cores, ep // 16, 16).transpose(0, 2, 1), (1, 8, 1))
    )
    # one-hot index table: position i -> [i%128, i//128]
    dloc_t = np.ascontiguousarray(
        dloc.reshape(cores, ct, P).transpose(0, 2, 1)
    ).astype(np.float32)

    # per-core dinv tables [128, nblk]
    dinv_tab = np.ones((cores, P, nblk), dtype=np.float32)
    dinvsq_tab = np.ones((cores, P, nblk), dtype=np.float32)
    for c in range(cores):
        d = dinv[c * ns : (c + 1) * ns]
        pad = np.ones(nblk * P - ns, dtype=np.float32)
        dp = np.concatenate([d, pad]).reshape(nblk, P).T
        dinv_tab[c] = dp
        dinvsq_tab[c] = dp * dp

    x = np.asarray(x, dtype=np.float32)
    W1b = np.asarray(W1, dtype=np.float32).astype(bf16)
    W2b = np.asarray(W2, dtype=np.float32).astype(bf16)
    b1r = np.broadcast_to(np.asarray(b1, np.float32), (P, W1b.shape[1])).copy()
    b2r = np.broadcast_to(np.asarray(b2, np.float32), (P, W2b.shape[1])).copy()

    in_maps = []
    for c in range(cores):
        in_maps.append(
            {
                "x": np.ascontiguousarray(x[c * ns : (c + 1) * ns]).astype(bf16),
                "w1": W1b,
                "w2": W2b,
                "b1": b1r,
                "b2": b2r,
                "dinv": np.ascontiguousarray(dinv_tab[c]),
                "dinvsq": np.ascontiguousarray(dinvsq_tab[c]),
                "gidx": gidx_w[c],
                "dloc": dloc_t[c],
            }
        )
    layout = {
        "n": n,
        "cores": cores,
        "ns": ns,
        "nblk": nblk,
        "caps": [int(v) for v in caps],
        "ct": ct,
        "ep": ep,
    }
    return in_maps, layout


# ----------------------------------------------------------------------------
# Device kernel builder
# ----------------------------------------------------------------------------


def build(layout, f_in=F_IN, h=H, c_out=C, reps=1):
    from concourse import bacc, mybir, tile
    from concourse.masks import make_identity

    n = layout["n"]
    cores = layout["cores"]
    ns = layout["ns"]
    nblk = layout["nblk"]
    caps = layout["caps"]
    ct = layout["ct"]
    ep = layout["ep"]
    kc1 = f_in // P
    kc2 = h // P
    zpad = P  # padded layer-2 row width (256B bf16)
    starts = [0]
    for cap in caps:
        starts.append(starts[-1] + cap)

    bf = mybir.dt.bfloat16
    f32 = mybir.dt.float32

    nc = bacc.Bacc(
        "TRN2",
        target_bir_lowering=False,
        debug=False,
        num_devices=cores,
        num_swdge_queues=int(os.environ.get("GCN_NQ", "1")),
        dynamic_dma_scratch_size=int(os.environ.get("GCN_SCR", "16384")),
    )
    x_in = nc.dram_tensor("x", [ns, f_in], bf, kind="ExternalInput").ap()
    w1_in = nc.dram_tensor("w1", [f_in, h], bf, kind="ExternalInput").ap()
    w2_in = nc.dram_tensor("w2", [h, c_out], bf, kind="ExternalInput").ap()
    b1_in = nc.dram_tensor("b1", [P, h], f32, kind="ExternalInput").ap()
    b2_in = nc.dram_tensor("b2", [P, c_out], f32, kind="ExternalInput").ap()
    dinv_in = nc.dram_tensor("dinv", [P, nblk], f32, kind="ExternalInput").ap()
    dinvsq_in = nc.dram_tensor(
        "dinvsq", [P, nblk], f32, kind="ExternalInput"
    ).ap()
    gidx_in = nc.dram_tensor(
        "gidx", [P, ep // 16], mybir.dt.int16, kind="ExternalInput"
    ).ap()
    dloc_in = nc.dram_tensor("dloc", [P, ct], f32, kind="ExternalInput").ap()
    out_ext = nc.dram_tensor(
        "out", [ns, c_out], f32, kind="ExternalOutput"
    ).ap()

    last_rows = ns - (nblk - 1) * P  # rows in the final (partial) block
    phase = int(os.environ.get("GCN_PHASE", "4"))

    with tile.TileContext(nc) as tc:
        with (
            tc.tile_pool(name="const", bufs=1) as cpool,
            tc.tile_pool(name="xt", bufs=2) as xtp,
            tc.tile_pool(name="oh", bufs=2) as ohp,
            tc.tile_pool(name="stg", bufs=2) as stgp,
            tc.tile_pool(name="pt", bufs=2, space="PSUM") as ptp,
            tc.tile_pool(name="pg", bufs=2, space="PSUM") as pgp,
            tc.tile_pool(name="pa", bufs=2, space="PSUM") as pap,
            tc.tile_pool(name="dram", bufs=1, space="DRAM") as dram,
        ):
            # ---- constants / persistent state ----
            ident = cpool.tile([P, P], bf)
            make_identity(nc, ident[:])
            iota_i = cpool.tile([P, P], mybir.dt.int16)
            nc.gpsimd.iota(iota_i[:], pattern=[[1, P]], channel_multiplier=0)
            iota_b = cpool.tile([P, P], bf)
            nc.vector.tensor_copy(out=iota_b[:], in_=iota_i[:])

            w1_s = cpool.tile([P, kc1, h], bf)
            for k in range(kc1):
                nc.sync.dma_start(w1_s[:, k, :], w1_in[k * P : (k + 1) * P, :])
            w2_s = cpool.tile([P, kc2, c_out], bf)
            for k in range(kc2):
                nc.sync.dma_start(w2_s[:, k, :], w2_in[k * P : (k + 1) * P, :])
            b1_s = cpool.tile([P, h], f32)
            nc.sync.dma_start(b1_s[:], b1_in[:])
            b2_s = cpool.tile([P, c_out], f32)
            nc.sync.dma_start(b2_s[:], b2_in[:])
            dinv_s = cpool.tile([P, nblk], f32)
            nc.sync.dma_start(dinv_s[:], dinv_in[:])
            dinvsq_s = cpool.tile([P, nblk], f32)
            nc.sync.dma_start(dinvsq_s[:], dinvsq_in[:])
            gidx_s = cpool.tile([P, ep // 16], mybir.dt.int16)
            nc.sync.dma_start(gidx_s[:], gidx_in[:])
            dloc_sb = cpool.tile([P, ct], f32)
            nc.sync.dma_start(dloc_sb[:], dloc_in[:])

            x_s = cpool.tile([P, nblk, f_in], bf)
            if last_rows < P:
                nc.vector.memset(x_s[:, nblk - 1, :], 0.0)
            for I in range(nblk):
                rows = P if I < nblk - 1 else last_rows
                nc.sync.dma_start(
                    x_s[0:rows, I, :], x_in[I * P : I * P + rows, :]
                )

            u_s = cpool.tile([P, nblk, h], bf)
            hs_s = cpool.tile([P, nblk, h], bf)
            h1_s = cpool.tile([P, nblk, h], bf)
            h1t_s = cpool.tile([P, kc2, nblk * P], bf)
            zp_s = cpool.tile([P, nblk, zpad], bf)
            nc.vector.memset(zp_s[:], 0.0)
            zs_s = cpool.tile([P, nblk, c_out], bf)
            out_s = cpool.tile([P, nblk, c_out], f32)

            tp_w = max(f_in, kc2 * P)

            agg_mode = os.environ.get("GCN_AGG", "full")

            def aggregate(table, elem, f_use, post):
                """Per dst-block: gather edge rows, incidence-matmul the
                segment sum into fp32 PSUM, then run post(I, agg_ap)."""
                for I in range(nblk):
                    cap = caps[I]
                    c0 = starts[I]
                    stg = stgp.tile([P, cap, elem], bf, tag=f"stg{elem}")
                    spl = int(os.environ.get("GCN_SPL", "768"))
                    for g0 in range(0, cap * P, spl):
                        g1 = min(g0 + spl, cap * P)
                        nc.gpsimd.dma_gather(
                            stg[:, g0 // P : g1 // P, :],
                            table[:],
                            gidx_s[:, c0 * 8 + g0 // 16 : c0 * 8 + g1 // 16],
                            num_idxs=g1 - g0,
                            num_idxs_reg=g1 - g0,
                            elem_size=elem,
                            queue_num=(g0 // spl)
                            % int(os.environ.get("GCN_NQ", "1")),
                        )
                    pat = pap.tile([P, h], f32, space="PSUM", tag="a")
                    pa = pat[:, 0:f_use]
                    if agg_mode == "gather":
                        nc.tensor.matmul(
                            pa,
                            lhsT=ident[:],
                            rhs=stg[:, 0, 0:f_use],
                            start=True,
                            stop=True,
                        )
                        post(I, pa)
                        continue
                    oh = ohp.tile([P, cap, P], bf, tag="oh")
                    for t in range(cap):
                        nc.vector.tensor_scalar(
                            oh[:, t, :],
                            iota_b[:],
                            dloc_sb[:, c0 + t : c0 + t + 1],
                            None,
                            mybir.AluOpType.is_equal,
                        )
                    if agg_mode == "oh":
                        nc.tensor.matmul(
                            pa,
                            lhsT=oh[:, 0, :],
                            rhs=stg[:, 0, 0:f_use],
                            start=True,
                            stop=True,
                        )
                        post(I, pa)
                        continue
                    for t in range(cap):
                        nc.tensor.matmul(
                            pa,
                            lhsT=oh[:, t, :],
                            rhs=stg[:, t, 0:f_use],
                            start=(t == 0),
                            stop=(t == cap - 1),
                        )
                    post(I, pa)

            for rep in range(reps):
                u_bounce = dram.tile([ns, h], bf, name=f"u_bounce{rep}")
                u_table = dram.tile(
                    [n, h], bf, addr_space="Shared", name=f"u_table{rep}"
                )
                z_bounce = dram.tile([ns, zpad], bf, name=f"z_bounce{rep}")
                z_table = dram.tile(
                    [n, zpad], bf, addr_space="Shared", name=f"z_table{rep}"
                )
                # ---- layer 1 GEMM: h_pre = x @ W1 ----
                for I in range(nblk):
                    xt_ps = ptp.tile([P, tp_w], bf, space="PSUM", tag="t")
                    for k in range(kc1):
                        nc.tensor.transpose(
                            out=xt_ps[:, k * P : (k + 1) * P],
                            in_=x_s[:, I, k * P : (k + 1) * P],
                            identity=ident[:],
                        )
                    xt_s = xtp.tile([P, f_in], bf, tag="xt")
                    nc.vector.tensor_copy(
                        out=xt_s[:], in_=xt_ps[:, 0:f_in]
                    )
                    pg = pgp.tile([P, h], f32, space="PSUM", tag="g")
                    for k in range(kc1):
                        nc.tensor.matmul(
                            pg[:],
                            lhsT=xt_s[:, k * P : (k + 1) * P],
                            rhs=w1_s[:, k, :],
                            start=(k == 0),
                            stop=(k == kc1 - 1),
                        )
                    nc.vector.tensor_scalar_mul(
                        u_s[:, I, :], pg[:], dinv_s[:, I : I + 1]
                    )
                    nc.scalar.mul(
                        hs_s[:, I, :], pg[:], dinvsq_s[:, I : I + 1]
                    )

                # ---- AllGather u ----
                if phase >= 2:
                    for I in range(nblk):
                        rows = P if I < nblk - 1 else last_rows
                        nc.sync.dma_start(
                            u_bounce[I * P : I * P + rows, :],
                            u_s[0:rows, I, :],
                        )
                    nc.gpsimd.collective_compute(
                        "AllGather",
                        mybir.AluOpType.bypass,
                        replica_groups=[list(range(cores))],
                        ins=[u_bounce.opt()],
                        outs=[u_table.opt()],
                    )

                # ---- layer 1 aggregation ----
                def post1(I, pa):
                    nc.vector.scalar_tensor_tensor(
                        out=h1_s[:, I, :],
                        in0=pa,
                        scalar=dinv_s[:, I : I + 1],
                        in1=hs_s[:, I, :],
                        op0=mybir.AluOpType.mult,
                        op1=mybir.AluOpType.add,
                    )
                    nc.vector.tensor_tensor(
                        out=h1_s[:, I, :],
                        in0=h1_s[:, I, :],
                        in1=b1_s[:],
                        op=mybir.AluOpType.add,
                    )
                    nc.vector.tensor_scalar_max(
                        h1_s[:, I, :], h1_s[:, I, :], 0.0
                    )

                if phase >= 3:
                    aggregate(u_table, h, h, post1)
                else:
                    nc.vector.memset(h1_s[:], 0.0)

                if phase >= 4:
                    # ---- layer 2 GEMM: z = h1 @ W2, u2 = dinv*z ----
                    for I in range(nblk):
                        ht_ps = ptp.tile([P, tp_w], bf, space="PSUM", tag="t")
                        for k in range(kc2):
                            nc.tensor.transpose(
                                out=ht_ps[:, k * P : (k + 1) * P],
                                in_=h1_s[:, I, k * P : (k + 1) * P],
                                identity=ident[:],
                            )
                        for k in range(kc2):
                            nc.vector.tensor_copy(
                                out=h1t_s[:, k, I * P : (I + 1) * P],
                                in_=ht_ps[:, k * P : (k + 1) * P],
                            )
                    for I in range(nblk):
                        pzt = pgp.tile([P, h], f32, space="PSUM", tag="g")
                        pz = pzt[:, 0:c_out]
                        for k in range(kc2):
                            nc.tensor.matmul(
                                pz,
                                lhsT=h1t_s[:, k, I * P : (I + 1) * P],
                                rhs=w2_s[:, k, :],
                                start=(k == 0),
                                stop=(k == kc2 - 1),
                            )
                        nc.vector.tensor_scalar_mul(
                            zp_s[:, I, 0:c_out], pz, dinv_s[:, I : I + 1]
                        )
                        nc.scalar.mul(
                            zs_s[:, I, :], pz, dinvsq_s[:, I : I + 1]
                        )

                    # ---- AllGather z ----
                    for I in range(nblk):
                        rows = P if I < nblk - 1 else last_rows
                        nc.sync.dma_start(
                            z_bounce[I * P : I * P + rows, :],
                            zp_s[0:rows, I, :],
                        )
                    nc.gpsimd.collective_compute(
                        "AllGather",
                        mybir.AluOpType.bypass,
                        replica_groups=[list(range(cores))],
                        ins=[z_bounce.opt()],
                        outs=[z_table.opt()],
                    )

                    # ---- layer 2 aggregation ----
                    def post2(I, pa):
                        nc.vector.scalar_tensor_tensor(
                            out=out_s[:, I, :],
                            in0=pa,
                            scalar=dinv_s[:, I : I + 1],
                            in1=zs_s[:, I, :],
                            op0=mybir.AluOpType.mult,
                            op1=mybir.AluOpType.add,
                        )
                        nc.vector.tensor_tensor(
                            out=out_s[:, I, :],
                            in0=out_s[:, I, :],
                            in1=b2_s[:],
                            op=mybir.AluOpType.add,
                        )

                    aggregate(z_table, zpad, c_out, post2)
                else:
                    nc.vector.memset(out_s[:], 0.0)
                    if phase >= 3:
                        for I in range(nblk):
                            nc.vector.tensor_copy(
                                out=out_s[:, I, :], in_=h1_s[:, I, 0:c_out]
                            )
                    else:
                        for I in range(nblk):
                            nc.vector.tensor_copy(
                                out=out_s[:, I, :], in_=u_s[:, I, 0:c_out]
                            )

                # ---- write out ----
                for I in range(nblk):
                    rows = P if I < nblk - 1 else last_rows
                    nc.sync.dma_start(
                        out_ext[I * P : I * P + rows, :], out_s[0:rows, I, :]
                    )

    nc.compile()
    return nc


# ----------------------------------------------------------------------------
# Entry point
# ----------------------------------------------------------------------------

_CACHE = {}


def kernel(x, edge_index, W1, b1, W2, b2):
    in_maps, layout = preprocess(x, edge_index, W1, b1, W2, b2)
    key = (layout["ct"], tuple(layout["caps"]))
    if key not in _CACHE:
        nc = build(layout)
        from runner_inline import SpmdRunner

        _CACHE[key] = (nc, SpmdRunner(nc, layout["cores"]))
    nc, runner = _CACHE[key]
    concat_in = runner.prepare(in_maps)
    out_arrs = runner.run(concat_in)
    res = runner.results(out_arrs)
    ns = layout["ns"]
    out = np.empty((N, C), dtype=np.float32)
    for c in range(layout["cores"]):
        out[c * ns : (c + 1) * ns] = res[c]["out"]
    return out / np.float32(TEMPERATURE)


# Self-contained runner (mirrors concourse.bass2jax.run_bass_via_pjrt but
# keeps the jitted callable for reuse). Written as a sibling module string
# during development; inlined here for the final self-contained kernel.
import sys
import types

_runner_src = None
try:
    import runner as _runner_mod  # dev environment

    sys.modules["runner_inline"] = _runner_mod
except ImportError:  # harness environment: inline definition
    import jax
    from jax.experimental.shard_map import shard_map
    from jax.sharding import Mesh, PartitionSpec

    def _make_runner_module():
        from concourse import bass2jax, mybir
        from concourse.bass2jax import _bass_exec_p, partition_id_tensor

        mod = types.ModuleType("runner_inline")

        class SpmdRunner:
            def __init__(self, nc, n_cores=8):
                bass2jax.install_neuronx_cc_hook()
                self.nc = nc
                self.n_cores = n_cores
                in_names, out_names, out_avals, zero_outs = [], [], [], []
                partition_name = (
                    nc.partition_id_tensor.name
                    if nc.partition_id_tensor
                    else None
                )
                for alloc in nc.m.functions[0].allocations:
                    if not isinstance(alloc, mybir.MemoryLocationSet):
                        continue
                    name = alloc.memorylocations[0].name
                    if alloc.kind == "ExternalInput":
                        if name != partition_name:
                            in_names.append(name)
                    elif alloc.kind == "ExternalOutput":
                        shape = tuple(alloc.tensor_shape)
                        dtype = mybir.dt.np(alloc.dtype)
                        out_names.append(name)
                        out_avals.append(jax.core.ShapedArray(shape, dtype))
                        zero_outs.append(np.zeros(shape, dtype))
                self.in_names = in_names
                self.out_names = out_names
                self.out_avals = out_avals
                n_params = len(in_names)
                n_outs = len(out_avals)
                all_in_names = in_names + out_names
                if partition_name is not None:
                    all_in_names.append(partition_name)

                def _body(*args):
                    operands = list(args)
                    if partition_name is not None:
                        operands.append(partition_id_tensor())
                    outs = _bass_exec_p.bind(
                        *operands,
                        out_avals=tuple(out_avals),
                        in_names=tuple(all_in_names),
                        out_names=tuple(out_names),
                        lowering_input_output_aliases=(),
                        sim_require_finite=True,
                        sim_require_nnan=True,
                        nc=nc,
                    )
                    return tuple(outs)

                devices = jax.devices()[:n_cores]
                mesh = Mesh(np.asarray(devices), ("core",))
                in_specs = (PartitionSpec("core"),) * (n_params + n_outs)
                out_specs = (PartitionSpec("core"),) * n_outs
                self.fn = jax.jit(
                    shard_map(
                        _body,
                        mesh=mesh,
                        in_specs=in_specs,
                        out_specs=out_specs,
                        check_rep=False,
                    ),
                    keep_unused=True,
                )
                self._concat_zeros = [
                    np.zeros((n_cores * z.shape[0], *z.shape[1:]), z.dtype)
                    for z in zero_outs
                ]

            def prepare(self, in_maps):
                per_core = [
                    [np.asarray(m[name]) for name in self.in_names]
                    for m in in_maps
                ]
                return [
                    np.concatenate(
                        [per_core[c][i] for c in range(self.n_cores)], axis=0
                    )
                    for i in range(len(self.in_names))
                ]

            def run(self, concat_in):
                out_arrs = self.fn(*concat_in, *self._concat_zeros)
                jax.block_until_ready(out_arrs)
                return out_arrs

            def results(self, out_arrs):
                return [
                    {
                        name: np.asarray(out_arrs[i]).reshape(
                            self.n_cores, *self.out_avals[i].shape
                        )[c]
                        for i, name in enumerate(self.out_names)
                    }
                    for c in range(self.n_cores)
                ]

        mod.SpmdRunner = SpmdRunner
        return mod

    sys.modules["runner_inline"] = _make_runner_module()
